# revision 7
# baseline (speedup 1.0000x reference)
"""Trainium2 Bass kernel for nn_AdaptiveMultiScaleFusion (deformable-conv fusion).

Sharding: 8 cores = 4 samples x 2 image halves (rows 0-47 / 48-95).
Each core computes both deformable-conv scales for its half; the only
cross-core exchange is a 3x128 partial-sum AllReduce within each core
pair (SE pooling + global-pool means need full-image means).

Deform conv realized as tent-weighted fixed-shift accumulation:
  sampled_k[c,p] = sum_{sy,sx in {-1,0,1}} ty_sy(dy_k[p]) tx_sx(dx_k[p])
                   * x[c, p + (ki-1+sy)*W + (kj-1+sx)]
with ty_{-1}=relu(-d), ty_0=1-|d|, ty_{+1}=relu(d) after clamping d to
[-1,1] (exact bilinear for |d|<1; ~0.6% of offsets exceed 1 and are
clamped). Per-pixel masks are replicated across the 128 channel
partitions via a DRAM round trip (stride-0 broadcast read on the DMA),
then the modulation runs on the Vector engine in fp16 2x mode and the
tap contraction accumulates on the TensorEngine in PSUM.
"""
import sys

sys.path.insert(0, '/opt/trn_rl_repo')

import numpy as np

import concourse.bass as bass
import concourse.bacc as bacc
import concourse.mybir as mybir
import concourse.tile as tile
from concourse import tile_utils
from concourse.bass_utils import run_bass_kernel_spmd
from concourse.alu_op_type import AluOpType

tile_utils.max_sbuf_usage = 207 * 1024

F16 = mybir.dt.float16
F32 = mybir.dt.float32
AF = mybir.ActivationFunctionType

CH = 128
HH = 48
WD = 96
MG = 3
WH = HH + 2 * MG   # 54
WW = WD + 2 * MG   # 102
NPIX = HH * WD     # 4608
NQ = 4
QR = HH // NQ      # 12
QP = QR * WD       # 1152
NC8 = 12           # 4-row chunks

SHIFTS = [(sy, sx) for sy in (-2, -1, 0, 1, 2) for sx in (-2, -1, 0, 1, 2)
          if not (abs(sy) == 2 and abs(sx) == 2)]  # 21 products
NPROD = len(SHIFTS)
NE = 8             # tent/product eighth-chunks
EP = NPIX // NE    # 576


def build_kernel(repeat=1):
    nc = bacc.Bacc("TRN2", target_bir_lowering=False, debug=False,
                   num_devices=8)

    dp = nc.declare_dram_parameter
    xw = dp("xw", [CH, WH, WW], F16, isOutput=False)
    xws = dp("xws", [CH, WH, WW], F16, isOutput=False)
    ow = dp("ow", [CH, 9, 100], F16, isOutput=False)
    offb = dp("offb", [100, 1], F32, isOutput=False)
    dwt = dp("dwt", [CH, 2, 9, CH], F16, isOutput=False)
    db = dp("db", [CH, 2], F32, isOutput=False)
    crt = dp("crt", [CH, CH], F16, isOutput=False)
    crb = dp("crb", [CH, 1], F32, isOutput=False)
    wg1t = dp("wg1t", [CH, 2, 8], F32, isOutput=False)
    wg1b = dp("wg1b", [8, 1], F32, isOutput=False)
    wgd = dp("wgd", [8, 1], F32, isOutput=False)
    wgdb = dp("wgdb", [1, 1], F32, isOutput=False)
    gp1t = dp("gp1t", [CH, 64], F32, isOutput=False)
    gp1b = dp("gp1b", [64, 1], F32, isOutput=False)
    gp2t = dp("gp2t", [64, 64], F32, isOutput=False)
    gp2b = dp("gp2b", [64, 1], F32, isOutput=False)
    gp3t = dp("gp3t", [64, CH], F32, isOutput=False)
    gp3b = dp("gp3b", [CH, 1], F32, isOutput=False)
    out_d = dp("out", [CH, NPIX], F32, isOutput=True)

    with tile.TileContext(nc) as tc:
        with (
            tc.tile_pool(name="xbuf", bufs=1) as xbuf,
            tc.tile_pool(name="wbuf", bufs=1) as wbuf,
            tc.tile_pool(name="mid", bufs=2) as midp,     # 9.2KB tiles
            tc.tile_pool(name="big48", bufs=2) as bigp,
            tc.tile_pool(name="tentc", bufs=16) as tentc,   # 20.7KB tiles
            tc.tile_pool(name="tmp", bufs=3) as tmpp,
            tc.tile_pool(name="acc", bufs=3) as accp,
            tc.tile_pool(name="small", bufs=1) as smallp,
            tc.tile_pool(name="outb", bufs=1) as outbp,
            tc.tile_pool(name="psA", bufs=2, space="PSUM") as psA,
            tc.tile_pool(name="psS", bufs=3, space="PSUM") as psS,
            tc.tile_pool(name="psT", bufs=2, space="PSUM") as psT,
            tc.tile_pool(name="dram", bufs=1, space="DRAM") as dramp,
        ):
            for _rep in range(repeat):
                # ---------- phase 0: loads ----------
                xe = xbuf.tile([CH, WH, WW], F16, tag="xe")
                nc.sync.dma_start(xe[:], xw[:])
                xo = xbuf.tile([CH, WH, WW], F16, tag="xo")
                nc.sync.dma_start(xo[:], xws[:])

                ow_sb = wbuf.tile([CH, 9, 100], F16, tag="ow")
                nc.sync.dma_start(ow_sb[:], ow[:])
                dwt_sb = wbuf.tile([CH, 2, 9, CH], F16, tag="dwt")
                nc.sync.dma_start(dwt_sb[:], dwt[:])
                crt_sb = wbuf.tile([CH, CH], F16, tag="crt")
                nc.sync.dma_start(crt_sb[:], crt[:])

                def load_small(name, shape, handle):
                    t = smallp.tile(shape, F32, tag=name)
                    nc.sync.dma_start(t[:], handle[:])
                    return t

                offb_sb = load_small("offb", [100, 1], offb)
                db_sb = load_small("db", [CH, 2], db)
                crb_sb = load_small("crb", [CH, 1], crb)
                wg1t_sb = load_small("wg1t", [CH, 2, 8], wg1t)
                wg1b_sb = load_small("wg1b", [8, 1], wg1b)
                wgd_sb = load_small("wgd", [8, 1], wgd)
                wgdb_sb = load_small("wgdb", [1, 1], wgdb)
                gp1t_sb = load_small("gp1t", [CH, 64], gp1t)
                gp1b_sb = load_small("gp1b", [64, 1], gp1b)
                gp2t_sb = load_small("gp2t", [64, 64], gp2t)
                gp2b_sb = load_small("gp2b", [64, 1], gp2b)
                gp3t_sb = load_small("gp3t", [64, CH], gp3t)
                gp3b_sb = load_small("gp3b", [CH, 1], gp3b)

                prod_dram = dramp.tile([36, NPROD, NPIX], F16, tag="prod_dram")

                # ---------- phase 1a: offsets conv -> clamped dy/dx (f16)
                dyt = midp.tile([36, NPIX], F16, tag="mid")
                dxt = midp.tile([36, NPIX], F16, tag="mid")
                for c in range(NC8):
                    ps = psA.tile([100, 4, WD], F32, tag="convps")
                    for t in range(9):
                        ki, kj = t // 3, t % 3
                        rhs = xe[:, MG + ki - 1 + 4 * c: MG + ki + 3 + 4 * c,
                                 MG + kj - 1: MG + kj - 1 + WD]
                        nc.tensor.matmul(ps[:], ow_sb[:, t, :], rhs,
                                         start=(t == 0), stop=(t == 8))
                    for lo, tgt in ((0, dyt), (64, dxt)):
                        seg = tgt[:, 4 * WD * c: 4 * WD * (c + 1)]
                        nc.vector.tensor_scalar(
                            seg, ps[lo:lo + 36, :, :], offb_sb[lo:lo + 36, :],
                            1.9995, AluOpType.add, AluOpType.min)
                        nc.vector.tensor_scalar_max(seg, seg, -1.9995)

                # ---------- phase 1bc: 5-tap axis tents + products ----
                def build_tents(dsrc, sl, nm):
                    base = dsrc[:, sl]
                    p = tentc.tile([36, EP], F16, tag="tc", name=f"p{nm}")
                    nc.vector.tensor_scalar_max(p[:], base, 0.0)
                    m = tentc.tile([36, EP], F16, tag="tc", name=f"m{nm}")
                    nc.vector.tensor_scalar(m[:], base, -1.0, 0.0,
                                            AluOpType.mult, AluOpType.max)
                    ep_ = tentc.tile([36, EP], F16, tag="tc", name=f"e{nm}")
                    nc.vector.tensor_scalar(ep_[:], p[:], 1.0, 0.0,
                                            AluOpType.subtract, AluOpType.max)
                    em = tentc.tile([36, EP], F16, tag="tc", name=f"f{nm}")
                    nc.vector.tensor_scalar(em[:], m[:], 1.0, 0.0,
                                            AluOpType.subtract, AluOpType.max)
                    t1 = tentc.tile([36, EP], F16, tag="tc", name=f"g{nm}")
                    nc.vector.tensor_scalar(t1[:], base, 2.0, -1.0,
                                            AluOpType.subtract, AluOpType.mult)
                    nc.vector.tensor_tensor(t1[:], t1[:], p[:], AluOpType.min)
                    tm1 = tentc.tile([36, EP], F16, tag="tc", name=f"h{nm}")
                    nc.vector.tensor_scalar_add(tm1[:], base, 2.0)
                    nc.vector.tensor_tensor(tm1[:], tm1[:], m[:], AluOpType.min)
                    t0 = tentc.tile([36, EP], F16, tag="tc", name=f"i{nm}")
                    nc.vector.tensor_add(t0[:], p[:], m[:])
                    nc.vector.tensor_scalar(t0[:], t0[:], 1.0, -1.0,
                                            AluOpType.subtract, AluOpType.mult)
                    nc.vector.tensor_scalar_max(t0[:], t0[:], 0.0)
                    return {-2: em, -1: tm1, 0: t0, 1: t1, 2: ep_}

                for e in range(NE):
                    sl = slice(EP * e, EP * (e + 1))
                    ty = build_tents(dyt, sl, f"y{e}")
                    tx = build_tents(dxt, sl, f"x{e}")
                    pr = bigp.tile([36, NPROD, EP], F16, tag="b48",
                                   name=f"pr{e}")
                    for j, (sy, sx) in enumerate(SHIFTS):
                        nc.vector.tensor_mul(pr[:, j, :], ty[sy][:], tx[sx][:])
                    nc.sync.dma_start(prod_dram[:, :, sl], pr[:])

                # ---------- phase 2: deformable convs ----------
                s_sb = []
                s_part = []
                pd_base = prod_dram[:]
                for s in range(2):
                    s_res = midp.tile([CH, HH, WD], F16, tag="mid", name=f"s_res{s}")
                    partials = []
                    for q in range(NQ):
                        pss = [psS.tile([CH, 4, WD], F32, tag="dps",
                                        name=f"dps_{s}_{q}_{i}")
                               for i in range(3)]
                        for t in range(9):
                            row = 18 * s + t
                            src = bass.AP(
                                pd_base.tensor,
                                pd_base.offset + row * NPROD * NPIX + QP * q,
                                [[0, CH], [NPIX, NPROD], [1, QP]])
                            mr = bigp.tile([CH, NPROD, QR, WD], F16,
                                           tag="b48")
                            nc.sync.dma_start(mr[:], src)
                            ki, kj = t // 3, t % 3
                            acc = None
                            for j, (sy, sx) in enumerate(SHIFTS):
                                r0 = MG + ki - 1 + sy + QR * q
                                cc = MG + kj - 1 + sx
                                if cc % 2 == 0:
                                    xv = xe[:, r0:r0 + QR, cc:cc + WD]
                                else:
                                    xv = xo[:, r0:r0 + QR,
                                            cc - 1:cc - 1 + WD]
                                tmp = tmpp.tile([CH, QR, WD], F16, tag="tmp")
                                nc.vector.tensor_mul(tmp[:], xv,
                                                     mr[:, j, :, :])
                                if acc is None:
                                    acc = tmp
                                else:
                                    a2 = accp.tile([CH, QR, WD], F16,
                                                   tag="acc")
                                    nc.vector.tensor_add(a2[:], acc[:],
                                                         tmp[:])
                                    acc = a2
                            for i in range(3):
                                nc.tensor.matmul(
                                    pss[i][:], dwt_sb[:, s, t, :],
                                    acc[:, 4 * i: 4 * (i + 1), :],
                                    start=(t == 0), stop=(t == 8))
                        for i in range(3):
                            pa = smallp.tile([CH, 1], F32,
                                             tag=f"pa{s}_{q}_{i}")
                            nc.scalar.activation(
                                s_res[:, QR * q + 4 * i: QR * q + 4 * i + 4,
                                      :],
                                pss[i][:], AF.Identity,
                                bias=db_sb[:, s:s + 1], accum_out=pa[:])
                            partials.append(pa)
                    while len(partials) > 1:
                        nxt = []
                        for i in range(0, len(partials) - 1, 2):
                            o = smallp.tile(
                                [CH, 1], F32,
                                tag=f"red{s}_{len(partials)}_{i}")
                            nc.vector.tensor_add(o[:], partials[i][:],
                                                 partials[i + 1][:])
                            nxt.append(o)
                        if len(partials) % 2:
                            nxt.append(partials[-1])
                        partials = nxt
                    s_part.append(partials[0])
                    s_sb.append(s_res)

                # ---------- phase 3: means exchange + gating ----------
                xsum_r = smallp.tile([CH, HH], F32, tag="xsum_r")
                nc.vector.tensor_reduce(xsum_r[:],
                                        xe[:, MG:MG + HH, MG:MG + WD],
                                        mybir.AxisListType.X, AluOpType.add)
                xsum = smallp.tile([CH, 1], F32, tag="xsum")
                nc.vector.tensor_reduce(xsum[:], xsum_r[:],
                                        mybir.AxisListType.X, AluOpType.add)

                cc_in = dramp.tile([3, CH], F32, tag="cc_in")
                cc_out = dramp.tile([3, CH], F32, tag="cc_out")
                nc.sync.dma_start(cc_in[0, :], xsum[:, 0])
                nc.sync.dma_start(cc_in[1, :], s_part[0][:, 0])
                nc.sync.dma_start(cc_in[2, :], s_part[1][:, 0])
                nc.gpsimd.collective_compute(
                    "AllReduce", AluOpType.add,
                    replica_groups=[[0, 1], [2, 3], [4, 5], [6, 7]],
                    ins=[cc_in.opt()], outs=[cc_out.opt()])
                xsum_g = smallp.tile([CH, 1], F32, tag="xsum_g")
                nc.sync.dma_start(xsum_g[:, 0], cc_out[0, :])
                s0sum_g = smallp.tile([CH, 1], F32, tag="s0sum_g")
                nc.sync.dma_start(s0sum_g[:, 0], cc_out[1, :])
                s1sum_g = smallp.tile([CH, 1], F32, tag="s1sum_g")
                nc.sync.dma_start(s1sum_g[:, 0], cc_out[2, :])

                ps_h = psT.tile([8, 1], F32, tag="mv")
                nc.tensor.matmul(ps_h[:], wg1t_sb[:, 0, :], s0sum_g[:],
                                 start=True, stop=False)
                nc.tensor.matmul(ps_h[:], wg1t_sb[:, 1, :], s1sum_g[:],
                                 start=False, stop=True)
                h_sb = smallp.tile([8, 1], F32, tag="h_sb")
                nc.scalar.activation(h_sb[:], ps_h[:], AF.Relu,
                                     bias=wg1b_sb[:])
                ps_z = psT.tile([8, 1], F32, tag="mv")
                nc.tensor.matmul(ps_z[0:1, :], wgd_sb[:], h_sb[:],
                                 start=True, stop=True)
                wts0 = smallp.tile([1, 1], F32, tag="wts0")
                nc.scalar.activation(wts0[:], ps_z[0:1, :], AF.Sigmoid,
                                     bias=wgdb_sb[:])
                wts1 = smallp.tile([1, 1], F32, tag="wts1")
                nc.vector.tensor_scalar(wts1[:], wts0[:], 1.0, -1.0,
                                        AluOpType.subtract, AluOpType.mult)
                wts0b = smallp.tile([CH, 1], F32, tag="wts0b")
                nc.gpsimd.partition_broadcast(wts0b[:], wts0[:])
                wts1b = smallp.tile([CH, 1], F32, tag="wts1b")
                nc.gpsimd.partition_broadcast(wts1b[:], wts1[:])

                ps_g1 = psT.tile([64, 1], F32, tag="mv")
                nc.tensor.matmul(ps_g1[:], gp1t_sb[:], xsum_g[:],
                                 start=True, stop=True)
                g1_sb = smallp.tile([64, 1], F32, tag="g1_sb")
                nc.scalar.activation(g1_sb[:], ps_g1[:], AF.Relu,
                                     bias=gp1b_sb[:])
                ps_g2 = psT.tile([64, 1], F32, tag="mv")
                nc.tensor.matmul(ps_g2[:], gp2t_sb[:], g1_sb[:],
                                 start=True, stop=True)
                g2_sb = smallp.tile([64, 1], F32, tag="g2_sb")
                nc.scalar.activation(g2_sb[:], ps_g2[:], AF.Relu,
                                     bias=gp2b_sb[:])
                ps_g3 = psT.tile([CH, 1], F32, tag="mv")
                nc.tensor.matmul(ps_g3[:], gp3t_sb[:], g2_sb[:],
                                 start=True, stop=True)
                g_sb = smallp.tile([CH, 1], F32, tag="g_sb")
                nc.scalar.activation(g_sb[:], ps_g3[:], AF.Sigmoid,
                                     bias=gp3b_sb[:])

                # ---------- phase 4: cr conv + combine + store ----------
                for c in range(NC8):
                    ps_cr = psA.tile([CH, 4, WD], F32, tag="convps")
                    nc.tensor.matmul(
                        ps_cr[:], crt_sb[:],
                        xe[:, MG + 4 * c: MG + 4 * c + 4, MG:MG + WD],
                        start=True, stop=True)
                    crsb = outbp.tile([CH, 4, WD], F32, tag="crsb")
                    nc.scalar.activation(crsb[:], ps_cr[:], AF.Identity,
                                         bias=crb_sb[:])
                    u = outbp.tile([CH, 4, WD], F32, tag="u")
                    nc.vector.tensor_scalar_mul(
                        u[:], s_sb[0][:, 4 * c: 4 * (c + 1), :], wts0b[:])
                    v = outbp.tile([CH, 4, WD], F32, tag="v")
                    nc.vector.scalar_tensor_tensor(
                        v[:], s_sb[1][:, 4 * c: 4 * (c + 1), :], wts1b[:],
                        u[:], AluOpType.mult, AluOpType.add)
                    w = outbp.tile([CH, 4, WD], F32, tag="w")
                    nc.vector.scalar_tensor_tensor(
                        w[:], crsb[:], g_sb[:], v[:],
                        AluOpType.mult, AluOpType.add)
                    nc.sync.dma_start(
                        out_d[:, 4 * WD * c: 4 * WD * (c + 1)],
                        w[:])

    nc.compile()
    return nc


# ---------------- host side ----------------


def _prep_inputs(x, off_w0, off_b0, dw0, db0, off_w1, off_b1, dw1, db1,
                 wg_w1, wg_b1, wg_w2, wg_b2,
                 gp_w1, gp_b1, gp_w2, gp_b2, gp_w3, gp_b3,
                 cr_w, cr_b):
    B, C, H, W = x.shape
    npix_full = float(H * W)

    ow = np.zeros((C, 9, 100), np.float16)
    offb = np.zeros((100, 1), np.float32)
    for t in range(9):
        ki, kj = t // 3, t % 3
        for axis in range(2):
            for s, w_ in enumerate((off_w0, off_w1)):
                for tap in range(9):
                    j = 64 * axis + 18 * s + tap
                    ow[:, t, j] = w_[2 * tap + axis, :, ki, kj]
    for axis in range(2):
        for s, b_ in enumerate((off_b0, off_b1)):
            for tap in range(9):
                offb[64 * axis + 18 * s + tap, 0] = b_[2 * tap + axis]

    dwt = np.zeros((C, 2, 9, C), np.float16)
    for s, w_ in enumerate((dw0, dw1)):
        for t in range(9):
            ki, kj = t // 3, t % 3
            dwt[:, s, t, :] = w_[:, :, ki, kj].T
    dbv = np.stack([db0, db1], axis=1).astype(np.float32)

    common = dict(
        ow=ow, offb=offb, dwt=dwt, db=dbv,
        crt=np.ascontiguousarray(cr_w.T).astype(np.float16),
        crb=cr_b.reshape(C, 1).astype(np.float32),
        wg1t=np.stack([wg_w1[:, :C].T, wg_w1[:, C:].T],
                      axis=1).astype(np.float32) / npix_full,
        wg1b=wg_b1.reshape(8, 1).astype(np.float32),
        wgd=(wg_w2[0] - wg_w2[1]).reshape(8, 1).astype(np.float32),
        wgdb=np.array([[wg_b2[0] - wg_b2[1]]], np.float32),
        gp1t=(gp_w1.T / npix_full).astype(np.float32),
        gp1b=gp_b1.reshape(64, 1).astype(np.float32),
        gp2t=np.ascontiguousarray(gp_w2.T).astype(np.float32),
        gp2b=gp_b2.reshape(64, 1).astype(np.float32),
        gp3t=np.ascontiguousarray(gp_w3.T).astype(np.float32),
        gp3b=gp_b3.reshape(C, 1).astype(np.float32),
    )

    in_maps = []
    for core in range(8):
        b = core // 2
        half = core % 2
        r0 = half * HH
        pad = np.zeros((C, WH, WW), np.float32)
        lo = r0 - MG
        hi = r0 + HH + MG
        slo = max(lo, 0)
        shi = min(hi, H)
        pad[:, slo - lo: shi - lo, MG:MG + W] = x[b, :, slo:shi, :]
        xwin = pad.astype(np.float16)
        xsh = np.zeros_like(xwin)
        xsh[:, :, :-1] = xwin[:, :, 1:]
        m = dict(common)
        m["xw"] = xwin
        m["xws"] = xsh
        in_maps.append(m)
    return in_maps


_NC_CACHE = {}


def kernel(**inputs):
    inputs = {k: np.asarray(v) for k, v in inputs.items()}
    x = inputs["x"]
    B, C, H, W = x.shape
    in_maps = _prep_inputs(**inputs)
    if "nc" not in _NC_CACHE:
        _NC_CACHE["nc"] = build_kernel()
    nc = _NC_CACHE["nc"]
    res = run_bass_kernel_spmd(nc, in_maps, core_ids=list(range(8)))
    out = np.zeros((B, C, H, W), np.float32)
    for core in range(8):
        b = core // 2
        half = core % 2
        o = res.results[core]["out"].reshape(C, HH, W)
        out[b, :, half * HH:(half + 1) * HH, :] = o
    return out


# revision 13
# speedup vs baseline: 6.2366x; 6.2366x over previous
"""Trainium2 Bass kernel for nn_AdaptiveMultiScaleFusion (deformable-conv fusion).

Sharding: 8 cores = 4 samples x 2 image halves (rows 0-47 / 48-95).
Each core computes both deformable-conv scales for its half; the only
cross-core exchange is a 3x128 partial-sum AllReduce within each core
pair (SE pooling + global-pool means need full-image means).

Deform conv realized as tent-weighted fixed-shift accumulation:
  sampled_k[c,p] = sum_{sy,sx in {-1,0,1}} ty_sy(dy_k[p]) tx_sx(dx_k[p])
                   * x[c, p + (ki-1+sy)*W + (kj-1+sx)]
with ty_{-1}=relu(-d), ty_0=1-|d|, ty_{+1}=relu(d) after clamping d to
[-1,1] (exact bilinear for |d|<1; ~0.6% of offsets exceed 1 and are
clamped). Per-pixel masks are replicated across the 128 channel
partitions via a DRAM round trip (stride-0 broadcast read on the DMA),
then the modulation runs on the Vector engine in fp16 2x mode and the
tap contraction accumulates on the TensorEngine in PSUM.
"""
import sys

sys.path.insert(0, '/opt/trn_rl_repo')

import numpy as np

import concourse.bass as bass
import concourse.bacc as bacc
import concourse.mybir as mybir
import concourse.tile as tile
from concourse import tile_utils
from concourse.bass_utils import run_bass_kernel_spmd
from concourse.alu_op_type import AluOpType

tile_utils.max_sbuf_usage = 207 * 1024

F16 = mybir.dt.float16
F32 = mybir.dt.float32
AF = mybir.ActivationFunctionType

CH = 128
HH = 48
WD = 96
MG = 3
WH = HH + 2 * MG   # 54
WW = WD + 2 * MG   # 102
NPIX = HH * WD     # 4608
NQ = 4
QR = HH // NQ      # 12
QP = QR * WD       # 1152
NC8 = 12           # 4-row chunks

SHIFTS = [(sy, sx) for sy in (-2, -1, 0, 1, 2) for sx in (-2, -1, 0, 1, 2)
          if not (abs(sy) == 2 and abs(sx) == 2)]  # 21 products
NPROD = len(SHIFTS)
NE = 8             # tent/product eighth-chunks
EP = NPIX // NE    # 576


def build_kernel(repeat=1, ablate=()):
    nc = bacc.Bacc("TRN2", target_bir_lowering=False, debug=False,
                   num_devices=8)

    dp = nc.declare_dram_parameter
    xw = dp("xw", [CH, WH, WW], F16, isOutput=False)
    xws = dp("xws", [CH, WH, WW], F16, isOutput=False)
    ow = dp("ow", [CH, 9, 100], F16, isOutput=False)
    offb = dp("offb", [100, 1], F32, isOutput=False)
    dwt = dp("dwt", [CH, 2, 9, CH], F16, isOutput=False)
    db = dp("db", [CH, 2], F32, isOutput=False)
    crt = dp("crt", [CH, CH], F16, isOutput=False)
    crb = dp("crb", [CH, 1], F32, isOutput=False)
    wg1t = dp("wg1t", [CH, 2, 8], F32, isOutput=False)
    wg1b = dp("wg1b", [8, 1], F32, isOutput=False)
    wgd = dp("wgd", [8, 1], F32, isOutput=False)
    wgdb = dp("wgdb", [1, 1], F32, isOutput=False)
    gp1t = dp("gp1t", [CH, 64], F32, isOutput=False)
    gp1b = dp("gp1b", [64, 1], F32, isOutput=False)
    gp2t = dp("gp2t", [64, 64], F32, isOutput=False)
    gp2b = dp("gp2b", [64, 1], F32, isOutput=False)
    gp3t = dp("gp3t", [64, CH], F32, isOutput=False)
    gp3b = dp("gp3b", [CH, 1], F32, isOutput=False)
    out_d = dp("out", [CH, NPIX], F32, isOutput=True)

    with tile.TileContext(nc) as tc:
        with (
            tc.tile_pool(name="xbuf", bufs=1) as xbuf,
            tc.tile_pool(name="wbuf", bufs=1) as wbuf,
            tc.tile_pool(name="mid", bufs=2) as midp,     # 9.2KB tiles
            tc.tile_pool(name="big48", bufs=2) as bigp,
            tc.tile_pool(name="tentc", bufs=14) as tentc,   # 20.7KB tiles
            tc.tile_pool(name="tmp", bufs=6) as tmpp,
            tc.tile_pool(name="acc", bufs=1) as accp,
            tc.tile_pool(name="small", bufs=1) as smallp,
            tc.tile_pool(name="outb", bufs=1) as outbp,
            tc.tile_pool(name="psA", bufs=2, space="PSUM") as psA,
            tc.tile_pool(name="psS", bufs=3, space="PSUM") as psS,
            tc.tile_pool(name="psT", bufs=2, space="PSUM") as psT,
            tc.tile_pool(name="dram", bufs=1, space="DRAM") as dramp,
        ):
            for _rep in range(repeat):
                # ---------- phase 0: loads ----------
                xe = xbuf.tile([CH, WH, WW], F16, tag="xe")
                nc.sync.dma_start(xe[:], xw[:])
                xo = xbuf.tile([CH, WH, WW], F16, tag="xo")
                nc.sync.dma_start(xo[:], xws[:])

                ow_sb = wbuf.tile([CH, 9, 100], F16, tag="ow")
                nc.sync.dma_start(ow_sb[:], ow[:])
                dwt_sb = wbuf.tile([CH, 2, 9, CH], F16, tag="dwt")
                nc.sync.dma_start(dwt_sb[:], dwt[:])
                crt_sb = wbuf.tile([CH, CH], F16, tag="crt")
                nc.sync.dma_start(crt_sb[:], crt[:])

                def load_small(name, shape, handle):
                    t = smallp.tile(shape, F32, tag=name)
                    nc.sync.dma_start(t[:], handle[:])
                    return t

                offb_sb = load_small("offb", [100, 1], offb)
                db_sb = load_small("db", [CH, 2], db)
                crb_sb = load_small("crb", [CH, 1], crb)
                wg1t_sb = load_small("wg1t", [CH, 2, 8], wg1t)
                wg1b_sb = load_small("wg1b", [8, 1], wg1b)
                wgd_sb = load_small("wgd", [8, 1], wgd)
                wgdb_sb = load_small("wgdb", [1, 1], wgdb)
                gp1t_sb = load_small("gp1t", [CH, 64], gp1t)
                gp1b_sb = load_small("gp1b", [64, 1], gp1b)
                gp2t_sb = load_small("gp2t", [64, 64], gp2t)
                gp2b_sb = load_small("gp2b", [64, 1], gp2b)
                gp3t_sb = load_small("gp3t", [64, CH], gp3t)
                gp3b_sb = load_small("gp3b", [CH, 1], gp3b)

                prod_dram = dramp.tile([36, NPROD, NPIX], F16, tag="prod_dram")

                # ---------- phase 1a: offsets conv -> clamped dy/dx (f16)
                dyt = midp.tile([36, NPIX], F16, tag="mid")
                dxt = midp.tile([36, NPIX], F16, tag="mid")
                for c in range(NC8):
                    ps = psA.tile([100, 4, WD], F32, tag="convps")
                    for t in range(9):
                        ki, kj = t // 3, t % 3
                        rhs = xe[:, MG + ki - 1 + 4 * c: MG + ki + 3 + 4 * c,
                                 MG + kj - 1: MG + kj - 1 + WD]
                        nc.tensor.matmul(ps[:], ow_sb[:, t, :], rhs,
                                         start=(t == 0), stop=(t == 8))
                    for lo, tgt in ((0, dyt), (64, dxt)):
                        seg = tgt[:, 4 * WD * c: 4 * WD * (c + 1)]
                        nc.vector.tensor_scalar(
                            seg, ps[lo:lo + 36, :, :], offb_sb[lo:lo + 36, :],
                            1.9995, AluOpType.add, AluOpType.min)
                        nc.vector.tensor_scalar_max(seg, seg, -1.9995)

                # ---------- phase 1bc: 5-tap axis tents + products ----
                def build_tents(dsrc, sl, nm):
                    base = dsrc[:, sl]
                    p = tentc.tile([36, EP], F16, tag="tc", name=f"p{nm}")
                    nc.vector.tensor_scalar_max(p[:], base, 0.0)
                    m = tentc.tile([36, EP], F16, tag="tc", name=f"m{nm}")
                    nc.vector.tensor_scalar(m[:], base, -1.0, 0.0,
                                            AluOpType.mult, AluOpType.max)
                    ep_ = tentc.tile([36, EP], F16, tag="tc", name=f"e{nm}")
                    nc.vector.tensor_scalar(ep_[:], p[:], 1.0, 0.0,
                                            AluOpType.subtract, AluOpType.max)
                    em = tentc.tile([36, EP], F16, tag="tc", name=f"f{nm}")
                    nc.vector.tensor_scalar(em[:], m[:], 1.0, 0.0,
                                            AluOpType.subtract, AluOpType.max)
                    t1 = tentc.tile([36, EP], F16, tag="tc", name=f"g{nm}")
                    nc.vector.tensor_scalar(t1[:], base, 2.0, -1.0,
                                            AluOpType.subtract, AluOpType.mult)
                    nc.vector.tensor_tensor(t1[:], t1[:], p[:], AluOpType.min)
                    tm1 = tentc.tile([36, EP], F16, tag="tc", name=f"h{nm}")
                    nc.vector.tensor_scalar_add(tm1[:], base, 2.0)
                    nc.vector.tensor_tensor(tm1[:], tm1[:], m[:], AluOpType.min)
                    t0 = tentc.tile([36, EP], F16, tag="tc", name=f"i{nm}")
                    nc.vector.tensor_add(t0[:], p[:], m[:])
                    nc.vector.tensor_scalar(t0[:], t0[:], 1.0, -1.0,
                                            AluOpType.subtract, AluOpType.mult)
                    nc.vector.tensor_scalar_max(t0[:], t0[:], 0.0)
                    return {-2: em, -1: tm1, 0: t0, 1: t1, 2: ep_}

                for e in range(NE):
                    sl = slice(EP * e, EP * (e + 1))
                    ty = build_tents(dyt, sl, f"y{e}")
                    tx = build_tents(dxt, sl, f"x{e}")
                    pr = bigp.tile([36, NPROD, EP], F16, tag="b48",
                                   name=f"pr{e}")
                    for j, (sy, sx) in enumerate(SHIFTS):
                        nc.vector.tensor_mul(pr[:, j, :], ty[sy][:], tx[sx][:])
                    nc.sync.dma_start(prod_dram[:, :, sl], pr[:])

                # ---------- phase 2: deformable convs ----------
                s_sb = []
                s_part = []
                pd_base = prod_dram[:]
                for s in range(2):
                    s_res = midp.tile([CH, HH, WD], F16, tag="mid", name=f"s_res{s}")
                    partials = []
                    for q in range(NQ):
                        pss = [psS.tile([CH, 4, WD], F32, tag="dps",
                                        name=f"dps_{s}_{q}_{i}")
                               for i in range(3)]
                        HG = 7  # mask half-group size (21 = 3 groups)
                        for t in range(9):
                            row = 18 * s + t
                            ki, kj = t // 3, t % 3
                            for g0 in range(0, NPROD, HG):
                                ng = min(HG, NPROD - g0)
                                src = bass.AP(
                                    pd_base.tensor,
                                    pd_base.offset + (row * NPROD + g0)
                                    * NPIX + QP * q,
                                    [[0, CH], [NPIX, ng], [1, QP]])
                                mr = bigp.tile([CH, HG, QR, WD], F16,
                                               tag="b16", bufs=4,
                                               name=f"mr_{t}_{g0}")
                                nc.sync.dma_start(mr[:, :ng, :, :], src)
                                for jj in range(ng):
                                    j = g0 + jj
                                    sy, sx = SHIFTS[j]
                                    r0 = MG + ki - 1 + sy + QR * q
                                    cc = MG + kj - 1 + sx
                                    if cc % 2 == 0:
                                        xv = xe[:, r0:r0 + QR, cc:cc + WD]
                                    else:
                                        xv = xo[:, r0:r0 + QR,
                                                cc - 1:cc - 1 + WD]
                                    tmpt = tmpp.tile([CH, QR, WD], F16,
                                                     tag="tmp")
                                    nc.vector.tensor_mul(tmpt[:], xv,
                                                         mr[:, jj, :, :])
                                    for i in range(3):
                                        nc.tensor.matmul(
                                            pss[i][:], dwt_sb[:, s, t, :],
                                            tmpt[:, 4 * i: 4 * (i + 1), :],
                                            start=(t == 0 and j == 0),
                                            stop=(t == 8 and
                                                  j == len(SHIFTS) - 1))
                        for i in range(3):
                            if "no_matmul" in ablate:
                                nc.vector.memset(pss[i][:], 0.0)
                            pa = smallp.tile([CH, 1], F32,
                                             tag=f"pa{s}_{q}_{i}")
                            nc.scalar.activation(
                                s_res[:, QR * q + 4 * i: QR * q + 4 * i + 4,
                                      :],
                                pss[i][:], AF.Identity,
                                bias=db_sb[:, s:s + 1], accum_out=pa[:])
                            partials.append(pa)
                    while len(partials) > 1:
                        nxt = []
                        for i in range(0, len(partials) - 1, 2):
                            o = smallp.tile(
                                [CH, 1], F32,
                                tag=f"red{s}_{len(partials)}_{i}")
                            nc.vector.tensor_add(o[:], partials[i][:],
                                                 partials[i + 1][:])
                            nxt.append(o)
                        if len(partials) % 2:
                            nxt.append(partials[-1])
                        partials = nxt
                    s_part.append(partials[0])
                    s_sb.append(s_res)

                # ---------- phase 3: means exchange + gating ----------
                xsum_r = smallp.tile([CH, HH], F32, tag="xsum_r")
                nc.vector.tensor_reduce(xsum_r[:],
                                        xe[:, MG:MG + HH, MG:MG + WD],
                                        mybir.AxisListType.X, AluOpType.add)
                xsum = smallp.tile([CH, 1], F32, tag="xsum")
                nc.vector.tensor_reduce(xsum[:], xsum_r[:],
                                        mybir.AxisListType.X, AluOpType.add)

                cc_in = dramp.tile([3, CH], F32, tag="cc_in")
                cc_out = dramp.tile([3, CH], F32, tag="cc_out")
                nc.sync.dma_start(cc_in[0, :], xsum[:, 0])
                nc.sync.dma_start(cc_in[1, :], s_part[0][:, 0])
                nc.sync.dma_start(cc_in[2, :], s_part[1][:, 0])
                nc.gpsimd.collective_compute(
                    "AllReduce", AluOpType.add,
                    replica_groups=[[0, 1], [2, 3], [4, 5], [6, 7]],
                    ins=[cc_in.opt()], outs=[cc_out.opt()])
                xsum_g = smallp.tile([CH, 1], F32, tag="xsum_g")
                nc.sync.dma_start(xsum_g[:, 0], cc_out[0, :])
                s0sum_g = smallp.tile([CH, 1], F32, tag="s0sum_g")
                nc.sync.dma_start(s0sum_g[:, 0], cc_out[1, :])
                s1sum_g = smallp.tile([CH, 1], F32, tag="s1sum_g")
                nc.sync.dma_start(s1sum_g[:, 0], cc_out[2, :])

                ps_h = psT.tile([8, 1], F32, tag="mv")
                nc.tensor.matmul(ps_h[:], wg1t_sb[:, 0, :], s0sum_g[:],
                                 start=True, stop=False)
                nc.tensor.matmul(ps_h[:], wg1t_sb[:, 1, :], s1sum_g[:],
                                 start=False, stop=True)
                h_sb = smallp.tile([8, 1], F32, tag="h_sb")
                nc.scalar.activation(h_sb[:], ps_h[:], AF.Relu,
                                     bias=wg1b_sb[:])
                ps_z = psT.tile([8, 1], F32, tag="mv")
                nc.tensor.matmul(ps_z[0:1, :], wgd_sb[:], h_sb[:],
                                 start=True, stop=True)
                wts0 = smallp.tile([1, 1], F32, tag="wts0")
                nc.scalar.activation(wts0[:], ps_z[0:1, :], AF.Sigmoid,
                                     bias=wgdb_sb[:])
                wts1 = smallp.tile([1, 1], F32, tag="wts1")
                nc.vector.tensor_scalar(wts1[:], wts0[:], 1.0, -1.0,
                                        AluOpType.subtract, AluOpType.mult)
                wts0b = smallp.tile([CH, 1], F32, tag="wts0b")
                nc.gpsimd.partition_broadcast(wts0b[:], wts0[:])
                wts1b = smallp.tile([CH, 1], F32, tag="wts1b")
                nc.gpsimd.partition_broadcast(wts1b[:], wts1[:])

                ps_g1 = psT.tile([64, 1], F32, tag="mv")
                nc.tensor.matmul(ps_g1[:], gp1t_sb[:], xsum_g[:],
                                 start=True, stop=True)
                g1_sb = smallp.tile([64, 1], F32, tag="g1_sb")
                nc.scalar.activation(g1_sb[:], ps_g1[:], AF.Relu,
                                     bias=gp1b_sb[:])
                ps_g2 = psT.tile([64, 1], F32, tag="mv")
                nc.tensor.matmul(ps_g2[:], gp2t_sb[:], g1_sb[:],
                                 start=True, stop=True)
                g2_sb = smallp.tile([64, 1], F32, tag="g2_sb")
                nc.scalar.activation(g2_sb[:], ps_g2[:], AF.Relu,
                                     bias=gp2b_sb[:])
                ps_g3 = psT.tile([CH, 1], F32, tag="mv")
                nc.tensor.matmul(ps_g3[:], gp3t_sb[:], g2_sb[:],
                                 start=True, stop=True)
                g_sb = smallp.tile([CH, 1], F32, tag="g_sb")
                nc.scalar.activation(g_sb[:], ps_g3[:], AF.Sigmoid,
                                     bias=gp3b_sb[:])

                # ---------- phase 4: cr conv + combine + store ----------
                for c in range(NC8):
                    ps_cr = psA.tile([CH, 4, WD], F32, tag="convps")
                    nc.tensor.matmul(
                        ps_cr[:], crt_sb[:],
                        xe[:, MG + 4 * c: MG + 4 * c + 4, MG:MG + WD],
                        start=True, stop=True)
                    crsb = outbp.tile([CH, 4, WD], F32, tag="crsb")
                    nc.scalar.activation(crsb[:], ps_cr[:], AF.Identity,
                                         bias=crb_sb[:])
                    u = outbp.tile([CH, 4, WD], F32, tag="u")
                    nc.vector.tensor_scalar_mul(
                        u[:], s_sb[0][:, 4 * c: 4 * (c + 1), :], wts0b[:])
                    v = outbp.tile([CH, 4, WD], F32, tag="v")
                    nc.vector.scalar_tensor_tensor(
                        v[:], s_sb[1][:, 4 * c: 4 * (c + 1), :], wts1b[:],
                        u[:], AluOpType.mult, AluOpType.add)
                    w = outbp.tile([CH, 4, WD], F32, tag="w")
                    nc.vector.scalar_tensor_tensor(
                        w[:], crsb[:], g_sb[:], v[:],
                        AluOpType.mult, AluOpType.add)
                    nc.sync.dma_start(
                        out_d[:, 4 * WD * c: 4 * WD * (c + 1)],
                        w[:])

    nc.compile()
    return nc


# ---------------- host side ----------------


def _prep_inputs(x, off_w0, off_b0, dw0, db0, off_w1, off_b1, dw1, db1,
                 wg_w1, wg_b1, wg_w2, wg_b2,
                 gp_w1, gp_b1, gp_w2, gp_b2, gp_w3, gp_b3,
                 cr_w, cr_b):
    B, C, H, W = x.shape
    npix_full = float(H * W)

    ow = np.zeros((C, 9, 100), np.float16)
    offb = np.zeros((100, 1), np.float32)
    for t in range(9):
        ki, kj = t // 3, t % 3
        for axis in range(2):
            for s, w_ in enumerate((off_w0, off_w1)):
                for tap in range(9):
                    j = 64 * axis + 18 * s + tap
                    ow[:, t, j] = w_[2 * tap + axis, :, ki, kj]
    for axis in range(2):
        for s, b_ in enumerate((off_b0, off_b1)):
            for tap in range(9):
                offb[64 * axis + 18 * s + tap, 0] = b_[2 * tap + axis]

    dwt = np.zeros((C, 2, 9, C), np.float16)
    for s, w_ in enumerate((dw0, dw1)):
        for t in range(9):
            ki, kj = t // 3, t % 3
            dwt[:, s, t, :] = w_[:, :, ki, kj].T
    dbv = np.stack([db0, db1], axis=1).astype(np.float32)

    common = dict(
        ow=ow, offb=offb, dwt=dwt, db=dbv,
        crt=np.ascontiguousarray(cr_w.T).astype(np.float16),
        crb=cr_b.reshape(C, 1).astype(np.float32),
        wg1t=np.stack([wg_w1[:, :C].T, wg_w1[:, C:].T],
                      axis=1).astype(np.float32) / npix_full,
        wg1b=wg_b1.reshape(8, 1).astype(np.float32),
        wgd=(wg_w2[0] - wg_w2[1]).reshape(8, 1).astype(np.float32),
        wgdb=np.array([[wg_b2[0] - wg_b2[1]]], np.float32),
        gp1t=(gp_w1.T / npix_full).astype(np.float32),
        gp1b=gp_b1.reshape(64, 1).astype(np.float32),
        gp2t=np.ascontiguousarray(gp_w2.T).astype(np.float32),
        gp2b=gp_b2.reshape(64, 1).astype(np.float32),
        gp3t=np.ascontiguousarray(gp_w3.T).astype(np.float32),
        gp3b=gp_b3.reshape(C, 1).astype(np.float32),
    )

    in_maps = []
    for core in range(8):
        b = core // 2
        half = core % 2
        r0 = half * HH
        pad = np.zeros((C, WH, WW), np.float32)
        lo = r0 - MG
        hi = r0 + HH + MG
        slo = max(lo, 0)
        shi = min(hi, H)
        pad[:, slo - lo: shi - lo, MG:MG + W] = x[b, :, slo:shi, :]
        xwin = pad.astype(np.float16)
        xsh = np.zeros_like(xwin)
        xsh[:, :, :-1] = xwin[:, :, 1:]
        m = dict(common)
        m["xw"] = xwin
        m["xws"] = xsh
        in_maps.append(m)
    return in_maps


_NC_CACHE = {}


def kernel(**inputs):
    inputs = {k: np.asarray(v) for k, v in inputs.items()}
    x = inputs["x"]
    B, C, H, W = x.shape
    in_maps = _prep_inputs(**inputs)
    if "nc" not in _NC_CACHE:
        _NC_CACHE["nc"] = build_kernel()
    nc = _NC_CACHE["nc"]
    res = run_bass_kernel_spmd(nc, in_maps, core_ids=list(range(8)))
    out = np.zeros((B, C, H, W), np.float32)
    for core in range(8):
        b = core // 2
        half = core % 2
        o = res.results[core]["out"].reshape(C, HH, W)
        out[b, :, half * HH:(half + 1) * HH, :] = o
    return out
